# revision 1
# baseline (speedup 1.0000x reference)
"""Trainium2 Bass kernel for nn_CollectiveDecActorTaxi0Obs (gnn_message_passing).

Computes, for obs [32768, 48], per-zone dense heads W [81, 48, 5] (+bias b,
adjacency idx/mask [81, 5]):
    logits = einsum('bd,ndk->bnk', obs, W) + b ; masked softmax over k
    out[b, n, idx[n, k]] += probs[b, n, k]              -> [32768, 81, 81] f32

Strategy (pure data parallelism, 8 cores, batch-sharded 4096 rows each):
  All small operands (W, b, idx, mask) are folded on the host into constant
  matrices so the device only runs matmuls + exp + elementwise:
    - Wa [49, 448]:   W flattened to padded slot columns with a bias row
                      appended; masked slots get bias -1e9 (exp underflows to
                      exactly 0, matching the reference's where(mask>0,.,-1e9)).
    - ob_p [pw, 81]:  0/1 slot->zone map -> per-zone sums of exp (softmax den)
    - E [81, 448]:    expands per-zone reciprocal denom back to slot rows
    - S [128, 6561]:  0/1 selection matrix built from idx; the scatter into
                      the 81-wide adjacency vector IS a matmul probs @ S
                      (duplicate idx entries accumulate, like .at[].add).
  fp32 matmuls on TRN2 cost 2 weight passes x 2 cycles/col; bf16 costs 1 x 1.
  probs is split hi+lo into two bf16 tensors (x == hi + lo to ~2^-18 relative)
  that are STACKED on the contraction axis: since both multiply the same 0/1
  S matrix (exact in bf16), one K=128 bf16 matmul computes hi@S + lo@S at a
  quarter of the fp32 cost (matmul time scales with N only). The same split
  handles the recip-denominator expansion. The softmax denominator matmul
  stays fp32 for accuracy; its reciprocal runs on the vector engine.

  Slot layout: 14 scatter groups of 6 zones (30 slots; last group 3 zones),
  two groups -> one 64-row half-chunk [A|pad|B], two half-chunks -> one
  128-row pair for the fp32 logits/den stage. The split tiles pcat hold the
  half-chunk's hi rows at 0..63 and lo rows at 64..127, so every scatter
  matmul is a full-K (128) single pass whose unused rows hit zero S rows.

  Everything runs in a transposed layout (batch on the free dim) until the
  scatter matmul, whose PSUM output lands batch-on-partitions so dense
  [128, 6561] tiles stream to DRAM with unit-stride rows.
  The kernel is HBM-write-bound: 860 MB of output, ~107 MB/core, ~320 us
  at the ~358 GB/s per-core HBM limit.
"""

import os
import sys

sys.path.insert(0, "/opt/trn_rl_repo")

import numpy as np

NZ = 81          # zones
D = 48           # obs dim used
DA = D + 1       # + bias row
KADJ = 5         # adjacency slots per zone
NCORES = 8
BATCH = 32768
BLOC = BATCH // NCORES   # 4096 rows per core
BF = 512                 # batch free-dim block (matmul N limit for fp32 PSUM)
P = 128
NEG = np.float32(-1e9)

ZPG = 6                        # zones per scatter group (30 slots + 2 pad)
NGRP = 14                      # groups: 13x6 zones + 1x3 zones
GRP_NZ = [6] * 13 + [3]
GRP_COL = [486 * g for g in range(14)]          # output column offset
PW_PAIR = [128, 128, 128, 64]  # used rows per pair (pair 3 = one half-chunk)
PADW = 448                     # 3*128 + 64 packed columns

LAST_RESULTS = None


def _slot(n, k):
    """(zone, k) -> (pair, row_in_pair, halfchunk, row_in_halfchunk_hi)."""
    g = n // ZPG
    zz = n % ZPG
    hc = g // 2
    p = hc // 2
    row_hi = 32 * (g % 2) + KADJ * zz + k       # 0..61 within half-chunk
    row_pair = 64 * (hc % 2) + row_hi
    return p, row_pair, hc, row_hi


def _build_consts(W, b, idx, mask):
    import ml_dtypes

    bf = ml_dtypes.bfloat16
    W = np.asarray(W, np.float32)
    b = np.asarray(b, np.float32)
    idx = np.asarray(idx)
    mask = np.asarray(mask, np.float32)

    Wa = np.zeros((DA, PADW), np.float32)
    E = np.zeros((NZ, PADW), bf)
    ob = [np.zeros((PW_PAIR[p], NZ), np.float32) for p in range(4)]
    S = np.zeros((P, NZ * NZ), bf)

    for n in range(NZ):
        for k in range(KADJ):
            p, rp, hc, rh = _slot(n, k)
            col = 128 * p + rp
            if mask[n, k] > 0:
                Wa[:D, col] = W[n, :, k]
                Wa[D, col] = b[n, k]
            else:
                Wa[D, col] = NEG
            E[n, col] = 1.0
            ob[p][rp, n] = 1.0
            ocol = n * NZ + int(idx[n, k])
            S[rh, ocol] = 1.0        # hi rows
            S[64 + rh, ocol] = 1.0   # lo rows
    return Wa, E, ob, S


def _build_program(bloc):
    from concourse import bacc, mybir
    import concourse.tile as tile

    f32 = mybir.dt.float32
    bf16 = mybir.dt.bfloat16
    AF = mybir.ActivationFunctionType
    OP = mybir.AluOpType
    nc = bacc.Bacc("TRN2", target_bir_lowering=False, debug=False)

    xTa_d = nc.declare_dram_parameter("xTa", [DA, bloc], f32, isOutput=False)
    Wa_d = nc.declare_dram_parameter("Wa", [DA, PADW], f32, isOutput=False)
    E_d = nc.declare_dram_parameter("E", [NZ, PADW], bf16, isOutput=False)
    ob_d = [
        nc.declare_dram_parameter(f"ob{p}", [PW_PAIR[p], NZ], f32, isOutput=False)
        for p in range(4)
    ]
    S_d = nc.declare_dram_parameter("S", [P, NZ * NZ], bf16, isOutput=False)
    out_d = nc.declare_dram_parameter("out", [bloc, NZ * NZ], f32, isOutput=True)

    n_blk = bloc // BF
    n_sub = BF // P

    with tile.TileContext(nc) as tc:
        with (
            tc.tile_pool(name="const", bufs=1) as cpool,
            tc.tile_pool(name="work", bufs=2) as wpool,
            tc.tile_pool(name="outp", bufs=4) as opool,
            tc.tile_pool(name="ps_log", bufs=2, space="PSUM") as ps_log,
            tc.tile_pool(name="ps_den", bufs=1, space="PSUM") as ps_den,
            tc.tile_pool(name="ps_rf", bufs=2, space="PSUM") as ps_rf,
            tc.tile_pool(name="ps_sc", bufs=3, space="PSUM") as ps_sc,
        ):
            Wa_sb = cpool.tile([DA, PADW], f32, tag="Wa")
            nc.sync.dma_start(out=Wa_sb[:], in_=Wa_d[:])
            E_sb = cpool.tile([NZ, PADW], bf16, tag="E")
            nc.sync.dma_start(out=E_sb[:], in_=E_d[:])
            S_sb = cpool.tile([P, NZ * NZ], bf16, tag="S")
            nc.sync.dma_start(out=S_sb[:], in_=S_d[:])
            ob_sb = []
            for p in range(4):
                t = cpool.tile([PW_PAIR[p], NZ], f32, tag=f"ob{p}")
                nc.sync.dma_start(out=t[:], in_=ob_d[p][:])
                ob_sb.append(t)
            xTa_sb = cpool.tile([DA, bloc], f32, tag="xTa")
            nc.sync.dma_start(out=xTa_sb[:], in_=xTa_d[:])

            def emit_scatter(bs, pcat):
                for i in range(n_sub):
                    osb = opool.tile([P, NZ * NZ], f32, tag="osb")
                    for g in range(NGRP):
                        ncols = GRP_NZ[g] * NZ
                        colg = GRP_COL[g]
                        sc = ps_sc.tile([P, BF], f32, tag="scps")
                        nc.tensor.matmul(
                            sc[:, :ncols],
                            pcat[g // 2][:, i * P:(i + 1) * P],
                            S_sb[:, colg:colg + ncols],
                            start=True,
                            stop=True,
                        )
                        dst = osb[:, colg:colg + ncols]
                        if g % 5 < 3:
                            nc.scalar.copy(dst, sc[:, :ncols])
                        else:
                            nc.vector.tensor_copy(dst, sc[:, :ncols])
                    nc.sync.dma_start(
                        out=out_d[bs + i * P: bs + (i + 1) * P, :], in_=osb[:]
                    )

            prev = None
            for blk in range(n_blk):
                bs = blk * BF
                exT = []
                for p in range(4):
                    pw = PW_PAIR[p]
                    lg = ps_log.tile([P, BF], f32, tag="lg")
                    nc.tensor.matmul(
                        lg[:pw, :],
                        Wa_sb[:, 128 * p:128 * p + pw],
                        xTa_sb[:, bs:bs + BF],
                        start=True,
                        stop=True,
                    )
                    ex = wpool.tile([P, BF], f32, tag=f"exp{p}")
                    nc.scalar.activation(ex[:pw, :], lg[:pw, :], AF.Exp)
                    exT.append(ex)
                den_ps = ps_den.tile([NZ, BF], f32, tag="den")
                for p in range(4):
                    nc.tensor.matmul(
                        den_ps[:, :], ob_sb[p][:], exT[p][:PW_PAIR[p], :],
                        start=(p == 0), stop=(p == 3),
                    )
                rc = wpool.tile([NZ, BF], f32, tag="recipC")
                nc.vector.reciprocal(rc[:], den_ps[:])
                rhi = wpool.tile([NZ, BF], bf16, tag="rhi")
                nc.scalar.copy(rhi[:], rc[:])
                rlo = wpool.tile([NZ, BF], bf16, tag="rlo")
                nc.vector.tensor_tensor(out=rlo[:], in0=rc[:], in1=rhi[:], op=OP.subtract)
                pcat = []
                for p in range(4):
                    pw = PW_PAIR[p]
                    rf = ps_rf.tile([P, BF], f32, tag="rf")
                    nc.tensor.matmul(
                        rf[:pw, :], E_sb[:, 128 * p:128 * p + pw], rhi[:],
                        start=True, stop=False,
                    )
                    nc.tensor.matmul(
                        rf[:pw, :], E_sb[:, 128 * p:128 * p + pw], rlo[:],
                        start=False, stop=True,
                    )
                    for h in range(2 if pw == 128 else 1):
                        sl = slice(64 * h, 64 * h + 64)
                        pt = wpool.tile([64, BF], f32, tag=f"pt{2 * p + h}")
                        nc.vector.tensor_tensor(
                            out=pt[:, :], in0=exT[p][sl, :], in1=rf[sl, :], op=OP.mult
                        )
                        pc = wpool.tile([P, BF], bf16, tag=f"pcat{2 * p + h}")
                        nc.scalar.copy(pc[:64, :], pt[:, :])
                        nc.vector.tensor_tensor(
                            out=pc[64:, :],
                            in0=pt[:, :],
                            in1=pc[:64, :],
                            op=OP.subtract,
                        )
                        pcat.append(pc)
                if prev is not None:
                    emit_scatter(*prev)
                prev = (bs, pcat)
            emit_scatter(*prev)
    nc.compile()
    return nc


def _install_ntff_hook():
    """Shim antenv.axon_hooks (absent in this image) so trace=True can drive
    NRT profiling through libaxon_pjrt.so. Only used for self-profiling."""
    import types

    try:
        import antenv

        try:
            from antenv.axon_hooks import get_axon_ntff_profile_hook  # noqa: F401

            return True
        except ImportError:
            pass
        if "/root/.axon_site" not in sys.path:
            sys.path.insert(0, "/root/.axon_site")
        from trn_agent_boot.trn_boot import _ntff_profile_via_ctypes

        hook = _ntff_profile_via_ctypes("/opt/axon/libaxon_pjrt.so")
        mod = types.ModuleType("antenv.axon_hooks")
        state = {"hook": hook}
        mod.get_axon_ntff_profile_hook = lambda: state["hook"]
        mod.set_axon_ntff_profile_hook = lambda h: state.update(hook=h)
        sys.modules["antenv.axon_hooks"] = mod
        antenv.axon_hooks = mod
        return hook is not None
    except Exception as e:  # profiling is best-effort; never break the run
        print("ntff hook install failed:", e)
        return False


def kernel(obs, W, b, idx, mask):
    from concourse.bass_utils import run_bass_kernel_spmd

    global LAST_RESULTS
    trace = bool(int(os.environ.get("KBT_TRACE", "0")))
    if trace:
        trace = _install_ntff_hook()
    obs = np.asarray(obs, np.float32)
    Wa, E, ob, S = _build_consts(W, b, idx, mask)

    nc = _build_program(BLOC)

    consts = {"Wa": Wa, "E": E, "S": S}
    for p in range(4):
        consts[f"ob{p}"] = ob[p]

    in_maps = []
    for i in range(NCORES):
        shard = obs[i * BLOC:(i + 1) * BLOC, :D]
        xTa = np.concatenate(
            [np.ascontiguousarray(shard.T), np.ones((1, BLOC), np.float32)], axis=0
        )
        m = dict(consts)
        m["xTa"] = np.ascontiguousarray(xTa)
        in_maps.append(m)

    br = run_bass_kernel_spmd(nc, in_maps, list(range(NCORES)), trace=trace)
    LAST_RESULTS = br
    out = np.concatenate([br.results[i]["out"] for i in range(NCORES)], axis=0)
    return out.reshape(BATCH, NZ, NZ)



# revision 2
# speedup vs baseline: 1.9681x; 1.9681x over previous
"""Trainium2 Bass kernel for nn_CollectiveDecActorTaxi0Obs (gnn_message_passing).

Computes, for obs [32768, 48], per-zone dense heads W [81, 48, 5] (+bias b,
adjacency idx/mask [81, 5]):
    logits = einsum('bd,ndk->bnk', obs, W) + b ; masked softmax over k
    out[b, n, idx[n, k]] += probs[b, n, k]              -> [32768, 81, 81] f32

Strategy (pure data parallelism, 8 cores, batch-sharded 4096 rows each):
  The kernel is HBM-write-bound: the only way below the f32 roofline
  (~107 MB/core ~ 300 us) is to store the output in fp16 (max abs storage
  error ~2^-11 on probs <= 1, vs the 2e-2 gate) and upcast on the host.
  That halves write traffic to ~54 MB/core (~150 us floor).

  Everything else is organized so the compute side stays far below the DMA
  floor.  Key observation: out[b, n*81 + idx[n,k]] with the 9x9 grid
  adjacency means idx[n,k] = n + delta, delta in {-9,-1,0,+1,+9}, so every
  non-zero output column is 82*n + delta: five stride-82 diagonals of the
  [81*81] row.  All other 6156 columns are exactly zero.

  Per 128-row batch tile (batch on partitions, no transposes anywhere):
    1. one fp32 matmul  lhsT=obsT[49,128] (bias via ones-row),
       rhs=Wa[49,405] -> logits [128, 405] in PSUM.  Wa packs W/b by
       slot = delta_set*81 + n; missing/masked slots get bias -1e9 so
       exp gives exactly 0.
    2. ScalarE exp -> ex [128, 405] f32 in SBUF.
    3. VectorE tensor_reduce over the 5 delta-sets (strided view) -> den,
       reciprocal_approx_fast -> rc (den is well-conditioned: >= exp(self
       logit) > 0).
    4. two VectorE tensor_tensor multiplies ex * rc -> fp16, writing
       DIRECTLY into the five diagonals of a pre-zeroed padded [128, 6592]
       fp16 tile via custom strided APs (delta sets {-9,0,9} share one AP
       with j-stride 9, sets {-1,+1} one with j-stride 2).  Masked slots
       write exp(-1e9)*rc = 0 onto columns that are true zeros (or pad).
    5. two such tiles -> one 3.3 MB DMA to DRAM (f16, near peak HBM BW).

  Engine budget per core: DMA ~150 us (bound), DVE ~55 us, PE ~26 us,
  Act ~16 us.
"""

import os
import sys

sys.path.insert(0, "/opt/trn_rl_repo")

import numpy as np

NZ = 81          # zones
GRID = 9
D = 48           # obs dim used
DA = D + 1       # + bias row
KADJ = 5         # adjacency slots per zone
NCORES = 8
BATCH = 32768
BLOC = BATCH // NCORES   # 4096 rows per core
P = 128
NSLOT = 5 * NZ           # 405 slot columns, delta-set major
NEG = np.float32(-1e9)

# delta-set order: groupA = {-9, 0, +9} (dst j-stride 9), groupB = {-1, +1}
DSETS = [-9, 0, 9, -1, 1]
DSET_IDX = {d: i for i, d in enumerate(DSETS)}
PAD = 16                 # left pad columns in the fp16 output tile
OSW = 6592               # padded tile width: PAD + 6561 + right pad, 32B rows
OW = NZ * NZ             # 6561
SUBS_PER_DMA = 2         # 256 batch rows -> 3.3 MB per DMA
NOSB = 3                 # output tile buffers

LAST_RESULTS = None


def _build_consts(W, b, idx, mask):
    W = np.asarray(W, np.float32)
    b = np.asarray(b, np.float32)
    idx = np.asarray(idx)
    mask = np.asarray(mask, np.float32)

    Wa = np.zeros((DA, NSLOT), np.float32)
    Wa[D, :] = NEG               # default: missing slot -> prob exactly 0
    seen = set()
    for n in range(NZ):
        for k in range(KADJ):
            if mask[n, k] <= 0:
                continue
            delta = int(idx[n, k]) - n
            assert delta in DSET_IDX, f"non-grid adjacency delta {delta}"
            col = DSET_IDX[delta] * NZ + n
            assert col not in seen
            seen.add(col)
            Wa[:D, col] = W[n, :, k]
            Wa[D, col] = b[n, k]
    return Wa


def _build_program(bloc):
    from concourse import bacc, mybir
    from concourse.ap import AP
    import concourse.tile as tile

    f32 = mybir.dt.float32
    f16 = mybir.dt.float16
    AF = mybir.ActivationFunctionType
    OP = mybir.AluOpType
    nc = bacc.Bacc("TRN2", target_bir_lowering=False, debug=False)

    xTa_d = nc.declare_dram_parameter("xTa", [DA, bloc], f32, isOutput=False)
    Wa_d = nc.declare_dram_parameter("Wa", [DA, NSLOT], f32, isOutput=False)
    out_d = nc.declare_dram_parameter("out", [bloc, OW], f16, isOutput=True)

    n_iter = bloc // (P * SUBS_PER_DMA)

    with tile.TileContext(nc) as tc:
        with (
            tc.tile_pool(name="const", bufs=1) as cpool,
            tc.tile_pool(name="work", bufs=3) as wpool,
            tc.tile_pool(name="den", bufs=2) as dpool,
            tc.tile_pool(name="ps_lg", bufs=3, space="PSUM") as ps_lg,
        ):
            Wa_sb = cpool.tile([DA, NSLOT], f32, tag="Wa")
            nc.sync.dma_start(out=Wa_sb[:], in_=Wa_d[:])
            xTa_sb = cpool.tile([DA, bloc], f32, tag="xTa")
            nc.sync.dma_start(out=xTa_sb[:], in_=xTa_d[:])

            osb = []
            for j in range(NOSB):
                t = cpool.tile([P, SUBS_PER_DMA * OSW], f16, tag=f"osb{j}")
                nc.vector.memset(t[:], 0.0)
                osb.append(t)

            def sb_view(t, col_off, dims):
                a = t[:]
                return AP(a.tensor, a.offset + col_off,
                          [list(a.ap[0])] + [[s, n] for s, n in dims])

            for it in range(n_iter):
                ob = osb[it % NOSB]
                for q in range(SUBS_PER_DMA):
                    c0 = (it * SUBS_PER_DMA + q) * P
                    lg = ps_lg.tile([P, NSLOT], f32, tag="lg")
                    nc.tensor.matmul(
                        lg[:], xTa_sb[:, c0:c0 + P], Wa_sb[:],
                        start=True, stop=True,
                    )
                    ex = wpool.tile([P, NSLOT], f32, tag="ex")
                    nc.scalar.activation(ex[:], lg[:], AF.Exp)
                    den = dpool.tile([P, NZ, 1], f32, tag="den")
                    nc.vector.tensor_reduce(
                        den[:],
                        ex[:].rearrange("p (j n) -> p j n", j=KADJ)
                             .transpose([0, 2, 1]),
                        mybir.AxisListType.X, OP.add,
                    )
                    rc = dpool.tile([P, NZ], f32, tag="rc")
                    nc.vector.reciprocal_approx_fast(
                        rc[:], den[:].squeeze(2))
                    base = q * OSW
                    # group A: delta {-9, 0, +9} -> dst col PAD-9+9j+82n
                    nc.vector.tensor_tensor(
                        out=sb_view(ob, base + PAD - 9, [[9, 3], [82, NZ]]),
                        in0=ex[:, 0:3 * NZ].rearrange("p (j n) -> p j n", j=3),
                        in1=rc[:].unsqueeze(1).broadcast_to([P, 3, NZ]),
                        op=OP.mult,
                    )
                    # group B: delta {-1, +1} -> dst col PAD-1+2j+82n
                    nc.vector.tensor_tensor(
                        out=sb_view(ob, base + PAD - 1, [[2, 2], [82, NZ]]),
                        in0=ex[:, 3 * NZ:5 * NZ].rearrange(
                            "p (j n) -> p j n", j=2),
                        in1=rc[:].unsqueeze(1).broadcast_to([P, 2, NZ]),
                        op=OP.mult,
                    )
                src = sb_view(ob, PAD, [[OSW, SUBS_PER_DMA], [1, OW]])
                oap = out_d[:]
                dst = AP(oap.tensor, it * SUBS_PER_DMA * P * OW,
                         [[OW, P], [P * OW, SUBS_PER_DMA], [1, OW]])
                nc.sync.dma_start(out=dst, in_=src)
    nc.compile()
    return nc


def _install_ntff_hook():
    """Shim antenv.axon_hooks (absent in this image) so trace=True can drive
    NRT profiling through libaxon_pjrt.so. Only used for self-profiling."""
    import types

    try:
        import antenv

        try:
            from antenv.axon_hooks import get_axon_ntff_profile_hook  # noqa: F401

            return True
        except ImportError:
            pass
        if "/root/.axon_site" not in sys.path:
            sys.path.insert(0, "/root/.axon_site")
        from trn_agent_boot.trn_boot import _ntff_profile_via_ctypes

        hook = _ntff_profile_via_ctypes("/opt/axon/libaxon_pjrt.so")
        mod = types.ModuleType("antenv.axon_hooks")
        state = {"hook": hook}
        mod.get_axon_ntff_profile_hook = lambda: state["hook"]
        mod.set_axon_ntff_profile_hook = lambda h: state.update(hook=h)
        sys.modules["antenv.axon_hooks"] = mod
        antenv.axon_hooks = mod
        return hook is not None
    except Exception as e:  # profiling is best-effort; never break the run
        print("ntff hook install failed:", e)
        return False


def kernel(obs, W, b, idx, mask):
    from concourse.bass_utils import run_bass_kernel_spmd

    global LAST_RESULTS
    trace = bool(int(os.environ.get("KBT_TRACE", "0")))
    if trace:
        trace = _install_ntff_hook()
    obs = np.asarray(obs, np.float32)
    Wa = _build_consts(W, b, idx, mask)

    nc = _build_program(BLOC)

    in_maps = []
    for i in range(NCORES):
        shard = obs[i * BLOC:(i + 1) * BLOC, :D]
        xTa = np.concatenate(
            [np.ascontiguousarray(shard.T), np.ones((1, BLOC), np.float32)],
            axis=0,
        )
        in_maps.append({"Wa": Wa, "xTa": np.ascontiguousarray(xTa)})

    br = run_bass_kernel_spmd(nc, in_maps, list(range(NCORES)), trace=trace)
    LAST_RESULTS = br
    out = np.concatenate(
        [br.results[i]["out"].astype(np.float32) for i in range(NCORES)],
        axis=0,
    )
    return out.reshape(BATCH, NZ, NZ)


# revision 5
# speedup vs baseline: 1.9915x; 1.0119x over previous
"""Trainium2 Bass kernel for nn_CollectiveDecActorTaxi0Obs (gnn_message_passing).

Computes, for obs [32768, 48], per-zone dense heads W [81, 48, 5] (+bias b,
adjacency idx/mask [81, 5]):
    logits = einsum('bd,ndk->bnk', obs, W) + b ; masked softmax over k
    out[b, n, idx[n, k]] += probs[b, n, k]              -> [32768, 81, 81] f32

Strategy (pure data parallelism, 8 cores, batch-sharded 4096 rows each):
  The kernel is HBM-write-bound: the only way below the f32 roofline
  (~107 MB/core ~ 300 us) is to store the output in fp16 (max abs storage
  error ~2^-11 on probs <= 1, vs the 2e-2 gate) and upcast on the host.
  That halves write traffic to ~54 MB/core (~150 us floor).

  Everything else is organized so the compute side stays far below the DMA
  floor.  Key observation: out[b, n*81 + idx[n,k]] with the 9x9 grid
  adjacency means idx[n,k] = n + delta, delta in {-9,-1,0,+1,+9}, so every
  non-zero output column is 82*n + delta: five stride-82 diagonals of the
  [81*81] row.  All other 6156 columns are exactly zero.

  Per 128-row batch tile (batch on partitions, no transposes anywhere):
    1. one fp32 matmul  lhsT=obsT[49,128] (bias via ones-row),
       rhs=Wa[49,405] -> logits [128, 405] in PSUM.  Wa packs W/b by
       slot = delta_set*81 + n; missing/masked slots get bias -1e9 so
       exp gives exactly 0.
    2. ScalarE exp -> ex [128, 405] f32 in SBUF.
    3. VectorE tensor_reduce over the 5 delta-sets (strided view) -> den,
       reciprocal_approx_fast -> rc (den is well-conditioned: >= exp(self
       logit) > 0).
    4. two VectorE tensor_tensor multiplies ex * rc -> fp16, writing
       DIRECTLY into the five diagonals of a pre-zeroed padded [128, 6592]
       fp16 tile via custom strided APs (delta sets {-9,0,9} share one AP
       with j-stride 9, sets {-1,+1} one with j-stride 2).  Masked slots
       write exp(-1e9)*rc = 0 onto columns that are true zeros (or pad).
    5. two such tiles -> one 3.3 MB DMA to DRAM (f16, near peak HBM BW).

  Engine budget per core: DMA ~150 us (bound), DVE ~55 us, PE ~26 us,
  Act ~16 us.
"""

import os
import sys

sys.path.insert(0, "/opt/trn_rl_repo")

import numpy as np

NZ = 81          # zones
GRID = 9
D = 48           # obs dim used
DA = D + 1       # + bias row
KADJ = 5         # adjacency slots per zone
NCORES = 8
BATCH = 32768
BLOC = BATCH // NCORES   # 4096 rows per core
P = 128
NSLOT = 5 * NZ           # 405 slot columns, delta-set major
NEG = np.float32(-1e9)

# delta-set order chosen so every SBUF store is a full-32-bit-word run:
# the probs tile ex holds [zeros(81) | d=-1 | d=0 | d=+1 | d=-9 | d=+9] and
# three tensor_tensor ops write (0,p-1,p0,p+1) at cols 82n-2..82n+1,
# (0,p-9) at 82n-10, (0,p+9) at 82n+8 -- all even-aligned, no partial-word
# read-modify-write.  The padding zeros land on provably-always-zero output
# columns (offsets {-2,-10,+8} mod 82 are never hot).
DSETS = [-1, 0, 1, -9, 9]
DSET_IDX = {d: i for i, d in enumerate(DSETS)}
EXW = NZ + NSLOT         # 486: leading zero block + 5 delta sets
PAD = 16                 # left pad columns in the fp16 output tile
OSW = 6592               # padded tile width: PAD + 6561 + right pad, 32B rows
OW = NZ * NZ             # 6561
SUBS_PER_DMA = 2         # 256 batch rows -> 3.3 MB per DMA
NOSB = 3                 # output tile buffers

LAST_RESULTS = None


def _build_consts(W, b, idx, mask):
    W = np.asarray(W, np.float32)
    b = np.asarray(b, np.float32)
    idx = np.asarray(idx)
    mask = np.asarray(mask, np.float32)

    Wa = np.zeros((DA, NSLOT), np.float32)
    Wa[D, :] = NEG               # default: missing slot -> prob exactly 0
    seen = set()
    for n in range(NZ):
        for k in range(KADJ):
            if mask[n, k] <= 0:
                continue
            delta = int(idx[n, k]) - n
            assert delta in DSET_IDX, f"non-grid adjacency delta {delta}"
            col = DSET_IDX[delta] * NZ + n
            assert col not in seen
            seen.add(col)
            Wa[:D, col] = W[n, :, k]
            Wa[D, col] = b[n, k]
    return Wa


def _build_program(bloc):
    from concourse import bacc, mybir
    from concourse.ap import AP
    import concourse.tile as tile

    f32 = mybir.dt.float32
    f16 = mybir.dt.float16
    AF = mybir.ActivationFunctionType
    OP = mybir.AluOpType
    nc = bacc.Bacc("TRN2", target_bir_lowering=False, debug=False)

    xTa_d = nc.declare_dram_parameter("xTa", [DA, bloc], f32, isOutput=False)
    Wa_d = nc.declare_dram_parameter("Wa", [DA, NSLOT], f32, isOutput=False)
    out_d = nc.declare_dram_parameter("out", [bloc, OW], f16, isOutput=True)

    n_iter = bloc // (P * SUBS_PER_DMA)

    with tile.TileContext(nc) as tc:
        with (
            tc.tile_pool(name="const", bufs=1) as cpool,
            tc.tile_pool(name="work", bufs=3) as wpool,
            tc.tile_pool(name="den", bufs=2) as dpool,
            tc.tile_pool(name="ps_lg", bufs=3, space="PSUM") as ps_lg,
        ):
            Wa_sb = cpool.tile([DA, NSLOT], f32, tag="Wa")
            nc.sync.dma_start(out=Wa_sb[:], in_=Wa_d[:])
            xTa_sb = cpool.tile([DA, bloc], f32, tag="xTa")
            nc.sync.dma_start(out=xTa_sb[:], in_=xTa_d[:])

            def sb_view(t, col_off, dims):
                a = t[:]
                return AP(a.tensor, a.offset + col_off,
                          [list(a.ap[0])] + [[s, n] for s, n in dims])

            osb = []
            for j in range(NOSB):
                t = cpool.tile([P, SUBS_PER_DMA * OSW], f16, tag=f"osb{j}")
                eng = nc.vector if j == 0 else nc.gpsimd
                eng.memset(t[:].bitcast(mybir.dt.int32), 0)
                osb.append(t)

            # pre-zero the leading zero block of the 3 rotating ex buffers
            ex_bufs = [wpool.tile([P, EXW], f32, tag="ex", name="ex")
                       for _ in range(3)]
            for t in ex_bufs:
                nc.vector.memset(t[:, :NZ], 0.0)

            for it in range(n_iter):
                ob = osb[it % NOSB]
                for q in range(SUBS_PER_DMA):
                    c0 = (it * SUBS_PER_DMA + q) * P
                    lg = ps_lg.tile([P, NSLOT], f32, tag="lg")
                    nc.tensor.matmul(
                        lg[:], xTa_sb[:, c0:c0 + P], Wa_sb[:],
                        start=True, stop=True,
                    )
                    ex = wpool.tile([P, EXW], f32, tag="ex")
                    nc.scalar.activation(ex[:, NZ:], lg[:], AF.Exp)
                    den = dpool.tile([P, NZ, 1], f32, tag="den")
                    nc.vector.tensor_reduce(
                        den[:],
                        ex[:, NZ:].rearrange("p (j n) -> p j n", j=KADJ)
                                  .transpose([0, 2, 1]),
                        mybir.AxisListType.X, OP.add,
                    )
                    rc = dpool.tile([P, NZ], f32, tag="rc")
                    nc.vector.reciprocal_approx_fast(
                        rc[:], den[:].squeeze(2))
                    base = q * OSW

                    def emit(dst_off, d_n, ex_stride):
                        nc.vector.tensor_tensor(
                            out=sb_view(ob, base + dst_off,
                                        [[82, NZ], [1, d_n]]),
                            in0=sb_view(ex, 0, [[1, NZ], [ex_stride, d_n]]),
                            in1=rc[:].unsqueeze(2).broadcast_to([P, NZ, d_n]),
                            op=OP.mult,
                        )

                    emit(PAD - 2, 4, NZ)         # (0, p[-1], p[0], p[+1])
                    emit(PAD - 10, 2, 4 * NZ)    # (0, p[-9])
                    emit(PAD + 8, 2, 5 * NZ)     # (0, p[+9])
                src = sb_view(ob, PAD, [[OSW, SUBS_PER_DMA], [1, OW]])
                oap = out_d[:]
                dst = AP(oap.tensor, it * SUBS_PER_DMA * P * OW,
                         [[OW, P], [P * OW, SUBS_PER_DMA], [1, OW]])
                nc.sync.dma_start(out=dst, in_=src)
    nc.compile()
    return nc


def _install_ntff_hook():
    """Shim antenv.axon_hooks (absent in this image) so trace=True can drive
    NRT profiling through libaxon_pjrt.so. Only used for self-profiling."""
    import types

    try:
        import antenv

        try:
            from antenv.axon_hooks import get_axon_ntff_profile_hook  # noqa: F401

            return True
        except ImportError:
            pass
        if "/root/.axon_site" not in sys.path:
            sys.path.insert(0, "/root/.axon_site")
        from trn_agent_boot.trn_boot import _ntff_profile_via_ctypes

        hook = _ntff_profile_via_ctypes("/opt/axon/libaxon_pjrt.so")
        mod = types.ModuleType("antenv.axon_hooks")
        state = {"hook": hook}
        mod.get_axon_ntff_profile_hook = lambda: state["hook"]
        mod.set_axon_ntff_profile_hook = lambda h: state.update(hook=h)
        sys.modules["antenv.axon_hooks"] = mod
        antenv.axon_hooks = mod
        return hook is not None
    except Exception as e:  # profiling is best-effort; never break the run
        print("ntff hook install failed:", e)
        return False


def kernel(obs, W, b, idx, mask):
    from concourse.bass_utils import run_bass_kernel_spmd

    global LAST_RESULTS
    trace = bool(int(os.environ.get("KBT_TRACE", "0")))
    if trace:
        trace = _install_ntff_hook()
    obs = np.asarray(obs, np.float32)
    Wa = _build_consts(W, b, idx, mask)

    nc = _build_program(BLOC)

    in_maps = []
    for i in range(NCORES):
        shard = obs[i * BLOC:(i + 1) * BLOC, :D]
        xTa = np.concatenate(
            [np.ascontiguousarray(shard.T), np.ones((1, BLOC), np.float32)],
            axis=0,
        )
        in_maps.append({"Wa": Wa, "xTa": np.ascontiguousarray(xTa)})

    br = run_bass_kernel_spmd(nc, in_maps, list(range(NCORES)), trace=trace)
    LAST_RESULTS = br
    out = np.concatenate(
        [br.results[i]["out"].astype(np.float32) for i in range(NCORES)],
        axis=0,
    )
    return out.reshape(BATCH, NZ, NZ)


# revision 9
# speedup vs baseline: 2.3929x; 1.2015x over previous
"""Trainium2 Bass kernel for nn_CollectiveDecActorTaxi0Obs (gnn_message_passing).

Computes, for obs [32768, 48], per-zone dense heads W [81, 48, 5] (+bias b,
adjacency idx/mask [81, 5]):
    logits = einsum('bd,ndk->bnk', obs, W) + b ; masked softmax over k
    out[b, n, idx[n, k]] += probs[b, n, k]              -> [32768, 81, 81] f32

Strategy (pure data parallelism, 8 cores, batch-sharded 4096 rows each):
  The kernel is HBM-write-bound: the only way below the f32 roofline
  (~107 MB/core ~ 300 us) is to store the output in fp16 (max abs storage
  error ~2^-11 on probs <= 1, vs the 2e-2 gate) and upcast on the host.
  That halves write traffic to ~54 MB/core (~150 us floor).

  Everything else is organized so the compute side stays far below the DMA
  floor.  Key observation: out[b, n*81 + idx[n,k]] with the 9x9 grid
  adjacency means idx[n,k] = n + delta, delta in {-9,-1,0,+1,+9}, so every
  non-zero output column is 82*n + delta: five stride-82 diagonals of the
  [81*81] row.  All other 6156 columns are exactly zero.

  Per 128-row batch tile (batch on partitions, no transposes anywhere):
    1. one fp32 matmul  lhsT=obsT[49,128] (bias via ones-row),
       rhs=Wa[49,405] -> logits [128, 405] in PSUM.  Wa packs W/b by
       slot = delta_set*81 + n; missing/masked slots get bias -1e9 so
       exp gives exactly 0.
    2. ScalarE exp -> ex [128, 405] f32 in SBUF.
    3. VectorE tensor_reduce over the 5 delta-sets (strided view) -> den,
       reciprocal_approx_fast -> rc (den is well-conditioned: >= exp(self
       logit) > 0).
    4. two VectorE tensor_tensor multiplies ex * rc -> fp16, writing
       DIRECTLY into the five diagonals of a pre-zeroed padded [128, 6592]
       fp16 tile via custom strided APs (delta sets {-9,0,9} share one AP
       with j-stride 9, sets {-1,+1} one with j-stride 2).  Masked slots
       write exp(-1e9)*rc = 0 onto columns that are true zeros (or pad).
    5. two such tiles -> one 3.3 MB DMA to DRAM (f16, near peak HBM BW).

  Engine budget per core: DMA ~150 us (bound), DVE ~55 us, PE ~26 us,
  Act ~16 us.
"""

import os
import sys

sys.path.insert(0, "/opt/trn_rl_repo")

import numpy as np

NZ = 81          # zones
GRID = 9
D = 48           # obs dim used
DA = D + 1       # + bias row
KADJ = 5         # adjacency slots per zone
NCORES = 8
BATCH = 32768
BLOC = BATCH // NCORES   # 4096 rows per core
P = 128
NSLOT = 5 * NZ           # 405 slot columns, delta-set major
NEG = np.float32(-1e9)

# delta-set order chosen so every SBUF store is a full-32-bit-word run:
# the probs tile ex holds [zeros(81) | d=-1 | d=0 | d=+1 | d=-9 | d=+9] and
# three tensor_tensor ops write (0,p-1,p0,p+1) at cols 82n-2..82n+1,
# (0,p-9) at 82n-10, (0,p+9) at 82n+8 -- all even-aligned, no partial-word
# read-modify-write.  The padding zeros land on provably-always-zero output
# columns (offsets {-2,-10,+8} mod 82 are never hot).
DSETS = [-1, 0, 1, -9, 9]
DSET_IDX = {d: i for i, d in enumerate(DSETS)}
EXW = NZ + NSLOT         # 486: leading zero block + 5 delta sets
PAD = 16                 # left pad columns in the fp16 output tile
OSW = 6592               # padded tile width: PAD + 6561 + right pad, 32B rows
OW = NZ * NZ             # 6561
SUBS_PER_DMA = 2         # 256 batch rows -> 3.3 MB per DMA
NOSB = 3                 # output tile buffers

LAST_RESULTS = None


def _build_consts(W, b, idx, mask):
    W = np.asarray(W, np.float32)
    b = np.asarray(b, np.float32)
    idx = np.asarray(idx)
    mask = np.asarray(mask, np.float32)

    Wa = np.zeros((DA, NSLOT), np.float32)
    Wa[D, :] = NEG               # default: missing slot -> prob exactly 0
    seen = set()
    for n in range(NZ):
        for k in range(KADJ):
            if mask[n, k] <= 0:
                continue
            delta = int(idx[n, k]) - n
            assert delta in DSET_IDX, f"non-grid adjacency delta {delta}"
            col = DSET_IDX[delta] * NZ + n
            assert col not in seen
            seen.add(col)
            Wa[:D, col] = W[n, :, k]
            Wa[D, col] = b[n, k]
    return Wa


def _build_program(bloc):
    from concourse import bacc, mybir
    from concourse.ap import AP
    import concourse.tile as tile

    f32 = mybir.dt.float32
    f16 = mybir.dt.float16
    AF = mybir.ActivationFunctionType
    OP = mybir.AluOpType
    nc = bacc.Bacc("TRN2", target_bir_lowering=False, debug=False)

    xTa_d = nc.declare_dram_parameter("xTa", [DA, bloc], f32, isOutput=False)
    Wa_d = nc.declare_dram_parameter("Wa", [DA, NSLOT], f32, isOutput=False)
    out_d = nc.declare_dram_parameter("out", [bloc, OW], f16, isOutput=True)
    NXCH = 4                     # JIT-load xTa in chunks; chunk 0 gates sub 0
    XCW = bloc // NXCH

    n_iter = bloc // (P * SUBS_PER_DMA)

    with tile.TileContext(nc) as tc:
        with (
            tc.tile_pool(name="const", bufs=1) as cpool,
            tc.tile_pool(name="work", bufs=3) as wpool,
            tc.tile_pool(name="den", bufs=2) as dpool,
            tc.tile_pool(name="ps_lg", bufs=3, space="PSUM") as ps_lg,
        ):
            def sb_view(t, col_off, dims):
                a = t[:]
                return AP(a.tensor, a.offset + col_off,
                          [list(a.ap[0])] + [[s, n] for s, n in dims])

            Wa_sb = cpool.tile([DA, NSLOT], f32, tag="Wa")
            nc.sync.dma_start(out=Wa_sb[:], in_=Wa_d[:])
            xch = []
            for j in range(NXCH):
                t = cpool.tile([DA, XCW], f32, tag=f"xch{j}", name="xch")
                nc.sync.dma_start(out=t[:], in_=xTa_d[:, j * XCW:(j + 1) * XCW])
                xch.append(t)

            # pre-zero the leading zero block of the 3 rotating ex buffers
            # (tiny, first on the DVE queue so nothing downstream waits)
            ex_bufs = [wpool.tile([P, EXW], f32, tag="ex", name="ex")
                       for _ in range(3)]
            for t in ex_bufs:
                nc.vector.memset(t[:, :NZ], 0.0)
            # preload the exp activation table with a dummy activation so the
            # ~2.7us ACT_TABLE_LOAD runs during the input DMA, not after it
            dum = dpool.tile([P, 1], f32, tag="dum")
            dum2 = dpool.tile([P, 1], f32, tag="dum2")
            nc.vector.memset(dum[:], 0.0)
            nc.scalar.activation(dum2[:], dum[:], AF.Exp)

            osb = []
            for j in range(NOSB):
                t = cpool.tile([P, SUBS_PER_DMA * OSW], f16, tag=f"osb{j}")
                nc.gpsimd.memset(t[:].bitcast(mybir.dt.int32), 0)
                osb.append(t)

            for it in range(n_iter):
                ob = osb[it % NOSB]
                for q in range(SUBS_PER_DMA):
                    s = it * SUBS_PER_DMA + q
                    xc = xch[s * P // XCW]
                    c0 = s * P % XCW
                    lg = ps_lg.tile([P, NSLOT], f32, tag="lg")
                    nc.tensor.matmul(
                        lg[:], xc[:, c0:c0 + P], Wa_sb[:],
                        start=True, stop=True,
                    )
                    ex = wpool.tile([P, EXW], f32, tag="ex")
                    nc.scalar.activation(ex[:, NZ:], lg[:], AF.Exp)
                    den = dpool.tile([P, NZ, 1], f32, tag="den")
                    nc.vector.tensor_reduce(
                        den[:],
                        ex[:, NZ:].rearrange("p (j n) -> p j n", j=KADJ)
                                  .transpose([0, 2, 1]),
                        mybir.AxisListType.X, OP.add,
                    )
                    rc = dpool.tile([P, NZ], f32, tag="rc")
                    nc.vector.reciprocal_approx_fast(
                        rc[:], den[:].squeeze(2))
                    base = q * OSW

                    def emit(dst_off, d_n, ex_stride):
                        nc.vector.tensor_tensor(
                            out=sb_view(ob, base + dst_off,
                                        [[82, NZ], [1, d_n]]),
                            in0=sb_view(ex, 0, [[1, NZ], [ex_stride, d_n]]),
                            in1=rc[:].unsqueeze(2).broadcast_to([P, NZ, d_n]),
                            op=OP.mult,
                        )

                    emit(PAD - 2, 4, NZ)         # (0, p[-1], p[0], p[+1])
                    emit(PAD - 10, 2, 4 * NZ)    # (0, p[-9])
                    emit(PAD + 8, 2, 5 * NZ)     # (0, p[+9])
                src = sb_view(ob, PAD, [[OSW, SUBS_PER_DMA], [1, OW]])
                oap = out_d[:]
                dst = AP(oap.tensor, it * SUBS_PER_DMA * P * OW,
                         [[OW, P], [P * OW, SUBS_PER_DMA], [1, OW]])
                nc.sync.dma_start(out=dst, in_=src)
    nc.compile()
    return nc


def _install_ntff_hook():
    """Shim antenv.axon_hooks (absent in this image) so trace=True can drive
    NRT profiling through libaxon_pjrt.so. Only used for self-profiling."""
    import types

    try:
        import antenv

        try:
            from antenv.axon_hooks import get_axon_ntff_profile_hook  # noqa: F401

            return True
        except ImportError:
            pass
        if "/root/.axon_site" not in sys.path:
            sys.path.insert(0, "/root/.axon_site")
        from trn_agent_boot.trn_boot import _ntff_profile_via_ctypes

        hook = _ntff_profile_via_ctypes("/opt/axon/libaxon_pjrt.so")
        mod = types.ModuleType("antenv.axon_hooks")
        state = {"hook": hook}
        mod.get_axon_ntff_profile_hook = lambda: state["hook"]
        mod.set_axon_ntff_profile_hook = lambda h: state.update(hook=h)
        sys.modules["antenv.axon_hooks"] = mod
        antenv.axon_hooks = mod
        return hook is not None
    except Exception as e:  # profiling is best-effort; never break the run
        print("ntff hook install failed:", e)
        return False


def kernel(obs, W, b, idx, mask):
    from concourse.bass_utils import run_bass_kernel_spmd

    global LAST_RESULTS
    trace = bool(int(os.environ.get("KBT_TRACE", "0")))
    if trace:
        trace = _install_ntff_hook()
    obs = np.asarray(obs, np.float32)
    Wa = _build_consts(W, b, idx, mask)

    nc = _build_program(BLOC)

    in_maps = []
    for i in range(NCORES):
        shard = obs[i * BLOC:(i + 1) * BLOC, :D]
        xTa = np.concatenate(
            [np.ascontiguousarray(shard.T), np.ones((1, BLOC), np.float32)],
            axis=0,
        )
        in_maps.append({"Wa": Wa, "xTa": np.ascontiguousarray(xTa)})

    br = run_bass_kernel_spmd(nc, in_maps, list(range(NCORES)), trace=trace)
    LAST_RESULTS = br
    out = np.concatenate(
        [br.results[i]["out"].astype(np.float32) for i in range(NCORES)],
        axis=0,
    )
    return out.reshape(BATCH, NZ, NZ)


# revision 11
# speedup vs baseline: 2.4352x; 1.0177x over previous
"""Trainium2 Bass kernel for nn_CollectiveDecActorTaxi0Obs (gnn_message_passing).

Computes, for obs [32768, 48], per-zone dense heads W [81, 48, 5] (+bias b,
adjacency idx/mask [81, 5]):
    logits = einsum('bd,ndk->bnk', obs, W) + b ; masked softmax over k
    out[b, n, idx[n, k]] += probs[b, n, k]              -> [32768, 81, 81] f32

Strategy (pure data parallelism, 8 cores, batch-sharded 4096 rows each):
  The kernel is HBM-write-bound: the only way below the f32 roofline
  (~107 MB/core ~ 300 us) is to store the output in fp16 (max abs storage
  error ~2^-11 on probs <= 1, vs the 2e-2 gate) and upcast on the host.
  That halves write traffic to ~54 MB/core (~150 us floor).

  Everything else is organized so the compute side stays far below the DMA
  floor.  Key observation: out[b, n*81 + idx[n,k]] with the 9x9 grid
  adjacency means idx[n,k] = n + delta, delta in {-9,-1,0,+1,+9}, so every
  non-zero output column is 82*n + delta: five stride-82 diagonals of the
  [81*81] row.  All other 6156 columns are exactly zero.

  Per 128-row batch tile (batch on partitions, no transposes anywhere):
    1. one fp32 matmul  lhsT=obsT[49,128] (bias via ones-row),
       rhs=Wa[49,405] -> logits [128, 405] in PSUM.  Wa packs W/b by
       slot = delta_set*81 + n; missing/masked slots get bias -1e9 so
       exp gives exactly 0.
    2. ScalarE exp -> ex [128, 405] f32 in SBUF.
    3. VectorE tensor_reduce over the 5 delta-sets (strided view) -> den,
       reciprocal_approx_fast -> rc (den is well-conditioned: >= exp(self
       logit) > 0).
    4. two VectorE tensor_tensor multiplies ex * rc -> fp16, writing
       DIRECTLY into the five diagonals of a pre-zeroed padded [128, 6592]
       fp16 tile via custom strided APs (delta sets {-9,0,9} share one AP
       with j-stride 9, sets {-1,+1} one with j-stride 2).  Masked slots
       write exp(-1e9)*rc = 0 onto columns that are true zeros (or pad).
    5. two such tiles -> one 3.3 MB DMA to DRAM (f16, near peak HBM BW).

  Engine budget per core: DMA ~150 us (bound), DVE ~55 us, PE ~26 us,
  Act ~16 us.
"""

import os
import sys

sys.path.insert(0, "/opt/trn_rl_repo")

import numpy as np

NZ = 81          # zones
GRID = 9
D = 48           # obs dim used
DA = D + 1       # + bias row
KADJ = 5         # adjacency slots per zone
NCORES = 8
BATCH = 32768
BLOC = BATCH // NCORES   # 4096 rows per core
P = 128
NSLOT = 5 * NZ           # 405 slot columns, delta-set major
NEG = np.float32(-1e9)

# delta-set order chosen so every SBUF store is a full-32-bit-word run:
# the probs tile ex holds [zeros(81) | d=-1 | d=0 | d=+1 | d=-9 | d=+9] and
# three tensor_tensor ops write (0,p-1,p0,p+1) at cols 82n-2..82n+1,
# (0,p-9) at 82n-10, (0,p+9) at 82n+8 -- all even-aligned, no partial-word
# read-modify-write.  The padding zeros land on provably-always-zero output
# columns (offsets {-2,-10,+8} mod 82 are never hot).
DSETS = [-1, 0, 1, -9, 9]
DSET_IDX = {d: i for i, d in enumerate(DSETS)}
EXW = NZ + NSLOT         # 486: leading zero block + 5 delta sets
PAD = 16                 # left pad columns in the fp16 output tile
OSW = 6592               # padded tile width: PAD + 6561 + right pad, 32B rows
OW = NZ * NZ             # 6561
SUBS_PER_DMA = 2         # 256 batch rows -> 3.3 MB per DMA
NOSB = 3                 # output tile buffers

LAST_RESULTS = None


def _build_consts(W, b, idx, mask):
    W = np.asarray(W, np.float32)
    b = np.asarray(b, np.float32)
    idx = np.asarray(idx)
    mask = np.asarray(mask, np.float32)

    Wa = np.zeros((DA, NSLOT), np.float32)
    Wa[D, :] = NEG               # default: missing slot -> prob exactly 0
    seen = set()
    for n in range(NZ):
        for k in range(KADJ):
            if mask[n, k] <= 0:
                continue
            delta = int(idx[n, k]) - n
            assert delta in DSET_IDX, f"non-grid adjacency delta {delta}"
            col = DSET_IDX[delta] * NZ + n
            assert col not in seen
            seen.add(col)
            Wa[:D, col] = W[n, :, k]
            Wa[D, col] = b[n, k]
    return Wa


def _build_program(bloc):
    from concourse import bacc, mybir
    from concourse.ap import AP
    import concourse.tile as tile

    f32 = mybir.dt.float32
    f16 = mybir.dt.float16
    AF = mybir.ActivationFunctionType
    OP = mybir.AluOpType
    nc = bacc.Bacc("TRN2", target_bir_lowering=False, debug=False)

    xTa_d = nc.declare_dram_parameter("xTa", [DA, bloc], f32, isOutput=False)
    Wa_d = nc.declare_dram_parameter("Wa", [DA, NSLOT], f32, isOutput=False)
    out_d = nc.declare_dram_parameter("out", [bloc, OW], f16, isOutput=True)
    NXCH = 8                     # JIT-load xTa in chunks; chunk 0 gates sub 0
    XCW = bloc // NXCH

    n_iter = bloc // (P * SUBS_PER_DMA)

    with tile.TileContext(nc) as tc:
        with (
            tc.tile_pool(name="const", bufs=1) as cpool,
            tc.tile_pool(name="work", bufs=3) as wpool,
            tc.tile_pool(name="den", bufs=2) as dpool,
            tc.tile_pool(name="ps_lg", bufs=3, space="PSUM") as ps_lg,
        ):
            def sb_view(t, col_off, dims):
                a = t[:]
                return AP(a.tensor, a.offset + col_off,
                          [list(a.ap[0])] + [[s, n] for s, n in dims])

            Wa_sb = cpool.tile([DA, NSLOT], f32, tag="Wa")
            nc.sync.dma_start(out=Wa_sb[:], in_=Wa_d[:])
            xch = []
            for j in range(NXCH):
                t = cpool.tile([DA, XCW], f32, tag=f"xch{j}", name="xch")
                nc.sync.dma_start(out=t[:], in_=xTa_d[:, j * XCW:(j + 1) * XCW])
                xch.append(t)

            # pre-zero the leading zero block of the 3 rotating ex buffers
            # (tiny, first on the DVE queue so nothing downstream waits)
            ex_bufs = [wpool.tile([P, EXW], f32, tag="ex", name="ex")
                       for _ in range(3)]
            for t in ex_bufs:
                nc.vector.memset(t[:, :NZ], 0.0)
            # preload the exp activation table with a dummy activation so the
            # ~2.7us ACT_TABLE_LOAD runs during the input DMA, not after it
            dum = dpool.tile([P, 1], f32, tag="dum")
            dum2 = dpool.tile([P, 1], f32, tag="dum2")
            nc.vector.memset(dum[:], 0.0)
            nc.scalar.activation(dum2[:], dum[:], AF.Exp)

            osb = []
            for j in range(NOSB):
                t = cpool.tile([P, SUBS_PER_DMA * OSW], f16, tag=f"osb{j}")
                nc.gpsimd.memset(t[:].bitcast(mybir.dt.int32), 0)
                osb.append(t)

            for it in range(n_iter):
                ob = osb[it % NOSB]
                for q in range(SUBS_PER_DMA):
                    s = it * SUBS_PER_DMA + q
                    xc = xch[s * P // XCW]
                    c0 = s * P % XCW
                    lg = ps_lg.tile([P, NSLOT], f32, tag="lg")
                    nc.tensor.matmul(
                        lg[:], xc[:, c0:c0 + P], Wa_sb[:],
                        start=True, stop=True,
                    )
                    ex = wpool.tile([P, EXW], f32, tag="ex")
                    nc.scalar.activation(ex[:, NZ:], lg[:], AF.Exp)
                    den = dpool.tile([P, NZ, 1], f32, tag="den")
                    nc.vector.tensor_reduce(
                        den[:],
                        ex[:, NZ:].rearrange("p (j n) -> p j n", j=KADJ)
                                  .transpose([0, 2, 1]),
                        mybir.AxisListType.X, OP.add,
                    )
                    rc = dpool.tile([P, NZ], f32, tag="rc")
                    nc.vector.reciprocal_approx_fast(
                        rc[:], den[:].squeeze(2))
                    base = q * OSW

                    def emit(dst_off, d_n, ex_stride):
                        nc.vector.tensor_tensor(
                            out=sb_view(ob, base + dst_off,
                                        [[82, NZ], [1, d_n]]),
                            in0=sb_view(ex, 0, [[1, NZ], [ex_stride, d_n]]),
                            in1=rc[:].unsqueeze(2).broadcast_to([P, NZ, d_n]),
                            op=OP.mult,
                        )

                    emit(PAD - 2, 4, NZ)         # (0, p[-1], p[0], p[+1])
                    emit(PAD - 10, 2, 4 * NZ)    # (0, p[-9])
                    emit(PAD + 8, 2, 5 * NZ)     # (0, p[+9])
                    if it < 2:
                        # early iterations: per-sub DMA so the write stream
                        # starts as soon as the first 128 rows are ready
                        src = sb_view(ob, q * OSW + PAD, [[1, OW]])
                        dst = AP(out_d[:].tensor,
                                 (it * SUBS_PER_DMA + q) * P * OW,
                                 [[OW, P], [1, OW]])
                        nc.sync.dma_start(out=dst, in_=src)
                if it >= 2:
                    src = sb_view(ob, PAD, [[OSW, SUBS_PER_DMA], [1, OW]])
                    dst = AP(out_d[:].tensor, it * SUBS_PER_DMA * P * OW,
                             [[OW, P], [P * OW, SUBS_PER_DMA], [1, OW]])
                    nc.sync.dma_start(out=dst, in_=src)
    nc.compile()
    return nc


def _install_ntff_hook():
    """Shim antenv.axon_hooks (absent in this image) so trace=True can drive
    NRT profiling through libaxon_pjrt.so. Only used for self-profiling."""
    import types

    try:
        import antenv

        try:
            from antenv.axon_hooks import get_axon_ntff_profile_hook  # noqa: F401

            return True
        except ImportError:
            pass
        if "/root/.axon_site" not in sys.path:
            sys.path.insert(0, "/root/.axon_site")
        from trn_agent_boot.trn_boot import _ntff_profile_via_ctypes

        hook = _ntff_profile_via_ctypes("/opt/axon/libaxon_pjrt.so")
        mod = types.ModuleType("antenv.axon_hooks")
        state = {"hook": hook}
        mod.get_axon_ntff_profile_hook = lambda: state["hook"]
        mod.set_axon_ntff_profile_hook = lambda h: state.update(hook=h)
        sys.modules["antenv.axon_hooks"] = mod
        antenv.axon_hooks = mod
        return hook is not None
    except Exception as e:  # profiling is best-effort; never break the run
        print("ntff hook install failed:", e)
        return False


def kernel(obs, W, b, idx, mask):
    from concourse.bass_utils import run_bass_kernel_spmd

    global LAST_RESULTS
    trace = bool(int(os.environ.get("KBT_TRACE", "0")))
    if trace:
        trace = _install_ntff_hook()
    obs = np.asarray(obs, np.float32)
    Wa = _build_consts(W, b, idx, mask)

    nc = _build_program(BLOC)

    in_maps = []
    for i in range(NCORES):
        shard = obs[i * BLOC:(i + 1) * BLOC, :D]
        xTa = np.concatenate(
            [np.ascontiguousarray(shard.T), np.ones((1, BLOC), np.float32)],
            axis=0,
        )
        in_maps.append({"Wa": Wa, "xTa": np.ascontiguousarray(xTa)})

    br = run_bass_kernel_spmd(nc, in_maps, list(range(NCORES)), trace=trace)
    LAST_RESULTS = br
    out = np.concatenate(
        [br.results[i]["out"].astype(np.float32) for i in range(NCORES)],
        axis=0,
    )
    return out.reshape(BATCH, NZ, NZ)
